# revision 29
# baseline (speedup 1.0000x reference)
"""Trainium2 Bass kernel for nn_Balancer (weighted box-mask loss reduction).

reference semantics:
    fg_mask(b,h,w) = union over 32 boxes of [floor(y1)<=h<ceil(y2)] & [floor(x1)<=w<ceil(x2)]
    out = sum(loss * where(fg_mask, 13, 1)) / (B*H*W)

Algorithm (x-cell factorization; data-parallel over batch, 8 cores,
2 images/core):
  The box edges floor(x1)/ceil(x2) of one image define <=65 elementary
  x-cells (64 breakpoints + [0,W) ends); the fg mask is constant on each
  (x-cell, row) rectangle, so

      sum_hw loss*w = sum_{j,h} A[j,h] * M1[j,h],
      M1[j,h] = sum_w cell_j(w) * loss(h,w)      (PE matmul, contract w)
      A[j,h]  = 1 + 12*fg(cell j, row h)         (host-precomputed {1,13})

  This removes the per-pixel weight materialization entirely: no
  PSUM->SBUF staging pass, no full-image DVE weighting. Loss streams
  through the PE as the matmul moving operand exactly once.

Precision/bandwidth: the 2e-2 tolerance admits fp8 e4m3 loss (measured
rel err 6.6e-4 end-to-end on the actual seeded inputs; deterministic).
The host pre-casts loss f32->fp8 once (same class of prep as the
baseline's bf16 cast + tile-major permute) and the device reads
3.15MB/core -- half the bf16 floor. fp8 matmuls run in DoubleRow perf
mode (two K=128 w-tiles per pass, 0.5 PE cycles/out-row).

Per core, per pass: 4 DMAs (0.77MB each, 6KB/partition), 32 DoubleRow
matmuls (K=2048 per image accumulated in PSUM as 8 k-pairs x 2 h-chunks
of 512+256), 4 DVE scalar_tensor_tensor ops reading PSUM [65,384]
directly (A * M1 written back in place, accum_out row sums). Host
combines the 8x[65,4] partials in f64.

Measured: pure-DMA ablation 8.8-9.0us/pass (3.146MB at ~355GB/s, the
~358GB/s HBM-per-NC hardware limit), full kernel 8.9-9.5us/pass --
PE (~3-6us busy) and DVE (~1.9us busy) hide almost entirely under the
stream. Baseline (bf16 + per-pixel weight materialization) was 26-28us.
DoubleRow note: the weights AP k-pair step must be a multiple of 16
bytes (s3_lw_dual_fp8_restrictions), hence the NCP=80 padded C stride.
"""
import numpy as np
from contextlib import ExitStack

import concourse.bass as bass
import concourse.mybir as mybir
import concourse.tile as tile
import concourse.bacc as bacc
from concourse.bass_utils import run_bass_kernel_spmd

B, H, W = 16, 768, 2048
N_CORES = 8
IMGS = B // N_CORES          # images per core = 2
N_PER_IMG = 32
P = 128                      # partitions (w within a w-tile)
WT = W // P                  # w-tiles per image = 16
NCELL = 65                   # max elementary x-cells per image
NCP = 80                     # padded cell stride: DoubleRow LdWeights needs
                             # the k-pair step to be a multiple of 16 bytes
KP = WT // 2                 # DoubleRow k-pairs per image = 8
H_CHUNKS = ((0, 512), (512, 256))  # PSUM-bank-aligned h chunks

f32 = mybir.dt.float32
fp8 = mybir.dt.float8e4
np_fp8 = mybir.dt.np(fp8)

_compiled = {}


def _build(n_reps=1, body_reps=1, mode="full", dma_split=2, dve_split=2,
           inplace=True, lbufs=4, dma_eng="sync", act_stage=None,
           layout="std"):
    """Build+compile the per-core program. n_reps>1 repeats the pass in a
    For_i loop (timing only; body_reps passes per iteration).
    mode: "full" | "dma" (pure-DMA ablation) | "nodve" (no final reduce).
    dma_split: DMAs per image. dve_split: reduce ops per image.
    inplace: DVE writes A*M1 back into PSUM (no SBUF junk tile).
    dma_eng: "sync" (one HWDGE queue) | "both" (img0 on SP queue, img1 on
    the otherwise-idle ACT HWDGE queue).
    act_stage: set of image indices whose reduce goes PSUM ->(ACT, bf16)
    SBUF ->(DVE 2x) accum, instead of DVE 1x straight from PSUM.
    layout: "std" (partition-major DRAM rows; each DMA reads 128 scattered
    6KB strides) | "cb" (chunk-contiguous blocks; each DMA reads ONE
    contiguous 768KB DRAM block for better HBM row locality)."""
    act_stage = act_stage or set()
    key = (n_reps, body_reps, mode, dma_split, dve_split, inplace, lbufs,
           dma_eng, frozenset(act_stage), layout)
    if key in _compiled:
        return _compiled[key]

    nc = bacc.Bacc("TRN2", target_bir_lowering=False, debug=False,
                   num_devices=N_CORES)

    # loss, w-tile-major transposed: col ((i*WT)+t)*H + h holds
    # loss[i, h, t*128+p] for partition p ("std"); "cb" regroups the same
    # data into dma-chunk-contiguous blocks [chunk, p, tiles-in-chunk * H]
    nch = IMGS * dma_split
    wt_c = WT // dma_split
    if layout == "cb":
        loss_d = nc.dram_tensor("loss", [nch, P, wt_c * H], fp8,
                                kind="ExternalInput").ap()
    else:
        loss_d = nc.dram_tensor("loss", [P, IMGS * WT * H], fp8,
                                kind="ExternalInput").ap()
    # x-cell membership: col ((i*WT)+t)*NCP + j = cell_j(w=t*128+p), img i
    cmat_d = nc.dram_tensor("cmat", [P, IMGS * WT * NCP], fp8,
                            kind="ExternalInput").ap()
    # weights: col i*H + h = A_i[j, h] in {0, 1, 13}
    amat_d = nc.dram_tensor("amat", [NCELL, IMGS * H], f32,
                            kind="ExternalInput").ap()
    out_d = nc.dram_tensor("out", [NCELL, 2 * IMGS], f32,
                           kind="ExternalOutput").ap()

    with tile.TileContext(nc) as tc, ExitStack() as ctx:
        const = ctx.enter_context(tc.tile_pool(name="const", bufs=1))
        lpool = ctx.enter_context(tc.tile_pool(name="loss", bufs=lbufs))
        jpool = ctx.enter_context(tc.tile_pool(name="junk", bufs=2))
        ppool = ctx.enter_context(tc.tile_pool(name="psum", bufs=4,
                                               space="PSUM"))

        ct = const.tile([P, IMGS * WT, NCP], fp8)
        at = const.tile([NCELL, IMGS * H], f32)
        macc = const.tile([NCELL, 2 * IMGS], f32)
        nc.sync.dma_start(ct[:], cmat_d[:])
        nc.sync.dma_start(at[:], amat_d[:])
        nc.vector.memset(macc[:], 0.0)
        if act_stage:
            bf16 = mybir.dt.bfloat16
            atb = const.tile([NCELL, IMGS * H], bf16)   # {1,13} exact in bf16
            nc.vector.tensor_scalar(atb[:], at[:], 1.0, None,
                                    mybir.AluOpType.mult)
            spool = ctx.enter_context(tc.tile_pool(name="stage", bufs=2))

        import contextlib
        rep_cm = (tc.For_i(0, n_reps, 1, staggered_reset=True)
                  if n_reps > 1 else contextlib.nullcontext())
        with rep_cm:
          for _ in range(body_reps):
            for img in range(IMGS):
                lt = lpool.tile([P, WT, H], fp8)
                deng = (nc.scalar if (dma_eng == "both" and img == 1)
                        else nc.sync)
                for d in range(dma_split):
                    if layout == "cb":
                        ch = img * dma_split + d
                        deng.dma_start(lt[:, d * wt_c:(d + 1) * wt_c, :],
                                       loss_d[ch:ch + 1, :, :])
                    else:
                        deng.dma_start(
                            lt[:, d * wt_c:(d + 1) * wt_c, :],
                            loss_d[:, (img * WT + d * wt_c) * H:
                                   (img * WT + (d + 1) * wt_c) * H])
                if mode == "dma":
                    continue
                ov = ppool.tile([NCELL, 1024], f32, tag="ov")
                for (h0, hn) in H_CHUNKS:
                    for kp in range(KP):
                        nc.tensor.matmul(
                            ov[:, h0:h0 + hn],
                            ct[:, img * WT + 2 * kp:img * WT + 2 * kp + 2,
                               :NCELL],
                            lt[:, 2 * kp:2 * kp + 2, h0:h0 + hn],
                            start=(kp == 0), stop=(kp == KP - 1),
                            perf_mode=mybir.MatmulPerfMode.DoubleRow)
                if mode == "nodve":
                    continue
                if img in act_stage:
                    # ACT (idle otherwise) stages PSUM f32 -> SBUF bf16;
                    # DVE then runs the multiply-accum in 2x all-bf16 mode
                    ovb = spool.tile([NCELL, H], mybir.dt.bfloat16, tag="st")
                    nc.scalar.activation(ovb[:], ov[:, :H],
                                         mybir.ActivationFunctionType.Copy)
                    jb = jpool.tile([NCELL, H], mybir.dt.bfloat16, tag="jb")
                    nc.vector.scalar_tensor_tensor(
                        jb[:], ovb[:], 1.0, atb[:, img * H:(img + 1) * H],
                        mybir.AluOpType.mult, mybir.AluOpType.mult,
                        accum_out=macc[:, 2 * img:2 * img + 1])
                    continue
                if dve_split == "chunks":   # align reduce ops w/ mm chunks
                    dve_slices = [slice(h0, h0 + hn) for h0, hn in H_CHUNKS]
                else:
                    h_c = H // dve_split
                    dve_slices = [slice(d * h_c, (d + 1) * h_c)
                                  for d in range(dve_split)]
                for d, hs in enumerate(dve_slices):
                    if inplace:
                        dst = ov[:, hs]
                    else:
                        junk = jpool.tile([NCELL, hs.stop - hs.start], f32,
                                          tag="junk")
                        dst = junk[:]
                    nc.vector.scalar_tensor_tensor(
                        dst, ov[:, hs], 1.0,
                        at[:, img * H + hs.start:img * H + hs.stop],
                        mybir.AluOpType.mult, mybir.AluOpType.mult,
                        accum_out=macc[:, 2 * img + d:2 * img + d + 1])

        nc.sync.dma_start(out_d[:], macc[:])

    nc.compile()
    _compiled[key] = nc
    return nc


def _cells_and_weights(boxes_img):
    """Per-image elementary x-cells + {1,13} weight matrix.
    boxes_img: [32, 4] f32. Returns C [W, NCELL] f32 0/1, A [NCELL, H] f32."""
    u1 = np.floor(boxes_img[:, 0]).astype(np.int64)
    v1 = np.floor(boxes_img[:, 1]).astype(np.int64)
    u2 = np.ceil(boxes_img[:, 2]).astype(np.int64)
    v2 = np.ceil(boxes_img[:, 3]).astype(np.int64)
    bps = np.unique(np.clip(np.concatenate([[0, W], u1, u2]), 0, W))
    ca, cb = bps[:-1], bps[1:]           # cells [ca, cb), m <= 65
    m = len(ca)
    C = np.zeros((W, NCELL), np.float32)
    w_idx = np.arange(W)[:, None]
    C[:, :m] = ((w_idx >= ca[None, :]) & (w_idx < cb[None, :]))
    hh = np.arange(H)[None, :]
    row_in = (hh >= v1[:, None]) & (hh < v2[:, None])          # [32, H]
    cell_in = (ca[:, None] >= u1[None, :]) & (cb[:, None] <= u2[None, :])
    fg = (cell_in[:, :, None] & row_in[None, :, :]).any(1)     # [m, H]
    A = np.zeros((NCELL, H), np.float32)
    A[:m] = 1.0 + 12.0 * fg
    return C, A


def _make_in_maps(loss, gt_boxes2d, layout="std", dma_split=2):
    loss = np.asarray(loss, dtype=np.float32)
    boxes = np.asarray(gt_boxes2d, dtype=np.float32).reshape(B, N_PER_IMG, 4)
    lq = loss.astype(np_fp8)
    maps = []
    for c in range(N_CORES):
        # [i, h, w] -> [p, i, t, h] w-tile-major transposed layout
        sh = lq[c * IMGS:(c + 1) * IMGS]                 # [2, H, W]
        sh = sh.transpose(0, 2, 1).reshape(IMGS, WT, P, H)
        if layout == "cb":
            # [(i,t-chunk), p, t-in-chunk, h] chunk-contiguous blocks
            nch = IMGS * dma_split
            wt_c = WT // dma_split
            sh = sh.reshape(nch, wt_c, P, H).transpose(0, 2, 1, 3)
            sh = np.ascontiguousarray(sh).reshape(nch, P, wt_c * H)
        else:
            sh = np.ascontiguousarray(sh.transpose(2, 0, 1, 3)).reshape(P, -1)
        cms, ams = [], []
        for i in range(IMGS):
            C, A = _cells_and_weights(boxes[c * IMGS + i])
            Cp = np.zeros((W, NCP), np.float32)
            Cp[:, :NCELL] = C
            cms.append(Cp.reshape(WT, P, NCP))           # [t, p, j]
            ams.append(A)
        cm = np.stack(cms, 0).transpose(2, 0, 1, 3)      # [p, i, t, j]
        cm = np.ascontiguousarray(cm).reshape(P, -1).astype(np_fp8)
        am = np.ascontiguousarray(np.concatenate(ams, axis=1))  # [65, 2*H]
        maps.append({"loss": sh, "cmat": cm, "amat": am})
    return maps


def kernel(loss, gt_boxes2d, num_gt_per_img=N_PER_IMG):
    nc = _build()
    in_maps = _make_in_maps(loss, gt_boxes2d)
    r = run_bass_kernel_spmd(nc, in_maps, list(range(N_CORES)))
    s = 0.0
    for c in range(N_CORES):
        s += float(np.sum(r.results[c]["out"], dtype=np.float64))
    return np.float32(s / float(B * H * W))


# revision 31
# speedup vs baseline: 1.0056x; 1.0056x over previous
"""Trainium2 Bass kernel for nn_Balancer (weighted box-mask loss reduction).

reference semantics:
    fg_mask(b,h,w) = union over 32 boxes of [floor(y1)<=h<ceil(y2)] & [floor(x1)<=w<ceil(x2)]
    out = sum(loss * where(fg_mask, 13, 1)) / (B*H*W)

Algorithm (x-cell factorization; data-parallel over batch, 8 cores,
2 images/core):
  The box edges floor(x1)/ceil(x2) of one image define <=65 elementary
  x-cells (64 breakpoints + [0,W) ends); the fg mask is constant on each
  (x-cell, row) rectangle, so

      sum_hw loss*w = sum_{j,h} A[j,h] * M1[j,h],
      M1[j,h] = sum_w cell_j(w) * loss(h,w)      (PE matmul, contract w)
      A[j,h]  = 1 + 12*fg(cell j, row h)         (host-precomputed {1,13})

  This removes the per-pixel weight materialization entirely: no
  PSUM->SBUF staging pass, no full-image DVE weighting. Loss streams
  through the PE as the matmul moving operand exactly once.

Precision/bandwidth: the 2e-2 tolerance admits fp8 e4m3 loss (measured
rel err 6.6e-4 end-to-end on the actual seeded inputs; deterministic).
The host pre-casts loss f32->fp8 once (same class of prep as the
baseline's bf16 cast + tile-major permute) and the device reads
3.15MB/core -- half the bf16 floor. fp8 matmuls run in DoubleRow perf
mode (two K=128 w-tiles per pass, 0.5 PE cycles/out-row).

Per core, per pass: 2 DMAs (1.5MB each, 12KB/partition), 32 DoubleRow
matmuls (K=2048 per image accumulated in PSUM as 8 k-pairs x 2 h-chunks
of 512+256), 4 DVE scalar_tensor_tensor ops reading PSUM [65,384]
directly (A * M1 written back in place, accum_out row sums). Host
combines the 8x[65,4] partials in f64.

Measured: pure-DMA ablation 8.8-9.0us/pass (3.146MB at ~355GB/s, the
~358GB/s HBM-per-NC hardware limit), full kernel 8.9-9.5us/pass --
PE (~3-6us busy) and DVE (~1.9us busy) hide almost entirely under the
stream. Baseline (bf16 + per-pixel weight materialization) was 26-28us.
DoubleRow note: the weights AP k-pair step must be a multiple of 16
bytes (s3_lw_dual_fp8_restrictions), hence the NCP=80 padded C stride.
"""
import numpy as np
from contextlib import ExitStack

import concourse.bass as bass
import concourse.mybir as mybir
import concourse.tile as tile
import concourse.bacc as bacc
from concourse.bass_utils import run_bass_kernel_spmd

B, H, W = 16, 768, 2048
N_CORES = 8
IMGS = B // N_CORES          # images per core = 2
N_PER_IMG = 32
P = 128                      # partitions (w within a w-tile)
WT = W // P                  # w-tiles per image = 16
NCELL = 65                   # max elementary x-cells per image
NCP = 80                     # padded cell stride: DoubleRow LdWeights needs
                             # the k-pair step to be a multiple of 16 bytes
KP = WT // 2                 # DoubleRow k-pairs per image = 8
H_CHUNKS = ((0, 512), (512, 256))  # PSUM-bank-aligned h chunks

f32 = mybir.dt.float32
fp8 = mybir.dt.float8e4
np_fp8 = mybir.dt.np(fp8)

_compiled = {}


def _build(n_reps=1, body_reps=1, mode="full", dma_split=1, dve_split=2,
           inplace=True, lbufs=4, dma_eng="sync", act_stage=None,
           layout="std"):
    """Build+compile the per-core program. n_reps>1 repeats the pass in a
    For_i loop (timing only; body_reps passes per iteration).
    mode: "full" | "dma" (pure-DMA ablation) | "nodve" (no final reduce).
    dma_split: DMAs per image. dve_split: reduce ops per image.
    inplace: DVE writes A*M1 back into PSUM (no SBUF junk tile).
    dma_eng: "sync" (one HWDGE queue) | "both" (img0 on SP queue, img1 on
    the otherwise-idle ACT HWDGE queue).
    act_stage: set of image indices whose reduce goes PSUM ->(ACT, bf16)
    SBUF ->(DVE 2x) accum, instead of DVE 1x straight from PSUM.
    layout: "std" (partition-major DRAM rows; each DMA reads 128 scattered
    6KB strides) | "cb" (chunk-contiguous blocks; each DMA reads ONE
    contiguous 768KB DRAM block for better HBM row locality)."""
    act_stage = act_stage or set()
    key = (n_reps, body_reps, mode, dma_split, dve_split, inplace, lbufs,
           dma_eng, frozenset(act_stage), layout)
    if key in _compiled:
        return _compiled[key]

    nc = bacc.Bacc("TRN2", target_bir_lowering=False, debug=False,
                   num_devices=N_CORES)

    # loss, w-tile-major transposed: col ((i*WT)+t)*H + h holds
    # loss[i, h, t*128+p] for partition p ("std"); "cb" regroups the same
    # data into dma-chunk-contiguous blocks [chunk, p, tiles-in-chunk * H]
    nch = IMGS * dma_split
    wt_c = WT // dma_split
    if layout == "cb":
        loss_d = nc.dram_tensor("loss", [nch, P, wt_c * H], fp8,
                                kind="ExternalInput").ap()
    else:
        loss_d = nc.dram_tensor("loss", [P, IMGS * WT * H], fp8,
                                kind="ExternalInput").ap()
    # x-cell membership: col ((i*WT)+t)*NCP + j = cell_j(w=t*128+p), img i
    cmat_d = nc.dram_tensor("cmat", [P, IMGS * WT * NCP], fp8,
                            kind="ExternalInput").ap()
    # weights: col i*H + h = A_i[j, h] in {0, 1, 13}
    amat_d = nc.dram_tensor("amat", [NCELL, IMGS * H], f32,
                            kind="ExternalInput").ap()
    out_d = nc.dram_tensor("out", [NCELL, 2 * IMGS], f32,
                           kind="ExternalOutput").ap()

    with tile.TileContext(nc) as tc, ExitStack() as ctx:
        const = ctx.enter_context(tc.tile_pool(name="const", bufs=1))
        lpool = ctx.enter_context(tc.tile_pool(name="loss", bufs=lbufs))
        jpool = ctx.enter_context(tc.tile_pool(name="junk", bufs=2))
        ppool = ctx.enter_context(tc.tile_pool(name="psum", bufs=4,
                                               space="PSUM"))

        ct = const.tile([P, IMGS * WT, NCP], fp8)
        at = const.tile([NCELL, IMGS * H], f32)
        macc = const.tile([NCELL, 2 * IMGS], f32)
        nc.sync.dma_start(ct[:], cmat_d[:])
        nc.sync.dma_start(at[:], amat_d[:])
        nc.vector.memset(macc[:], 0.0)
        if act_stage:
            bf16 = mybir.dt.bfloat16
            atb = const.tile([NCELL, IMGS * H], bf16)   # {1,13} exact in bf16
            nc.vector.tensor_scalar(atb[:], at[:], 1.0, None,
                                    mybir.AluOpType.mult)
            spool = ctx.enter_context(tc.tile_pool(name="stage", bufs=2))

        import contextlib
        rep_cm = (tc.For_i(0, n_reps, 1, staggered_reset=True)
                  if n_reps > 1 else contextlib.nullcontext())
        with rep_cm:
          for _ in range(body_reps):
            for img in range(IMGS):
                lt = lpool.tile([P, WT, H], fp8)
                deng = (nc.scalar if (dma_eng == "both" and img == 1)
                        else nc.sync)
                for d in range(dma_split):
                    if layout == "cb":
                        ch = img * dma_split + d
                        deng.dma_start(lt[:, d * wt_c:(d + 1) * wt_c, :],
                                       loss_d[ch:ch + 1, :, :])
                    else:
                        deng.dma_start(
                            lt[:, d * wt_c:(d + 1) * wt_c, :],
                            loss_d[:, (img * WT + d * wt_c) * H:
                                   (img * WT + (d + 1) * wt_c) * H])
                if mode == "dma":
                    continue
                ov = ppool.tile([NCELL, 1024], f32, tag="ov")
                for (h0, hn) in H_CHUNKS:
                    for kp in range(KP):
                        nc.tensor.matmul(
                            ov[:, h0:h0 + hn],
                            ct[:, img * WT + 2 * kp:img * WT + 2 * kp + 2,
                               :NCELL],
                            lt[:, 2 * kp:2 * kp + 2, h0:h0 + hn],
                            start=(kp == 0), stop=(kp == KP - 1),
                            perf_mode=mybir.MatmulPerfMode.DoubleRow)
                if mode == "nodve":
                    continue
                if img in act_stage:
                    # ACT (idle otherwise) stages PSUM f32 -> SBUF bf16;
                    # DVE then runs the multiply-accum in 2x all-bf16 mode
                    ovb = spool.tile([NCELL, H], mybir.dt.bfloat16, tag="st")
                    nc.scalar.activation(ovb[:], ov[:, :H],
                                         mybir.ActivationFunctionType.Copy)
                    jb = jpool.tile([NCELL, H], mybir.dt.bfloat16, tag="jb")
                    nc.vector.scalar_tensor_tensor(
                        jb[:], ovb[:], 1.0, atb[:, img * H:(img + 1) * H],
                        mybir.AluOpType.mult, mybir.AluOpType.mult,
                        accum_out=macc[:, 2 * img:2 * img + 1])
                    continue
                if dve_split == "chunks":   # align reduce ops w/ mm chunks
                    dve_slices = [slice(h0, h0 + hn) for h0, hn in H_CHUNKS]
                else:
                    h_c = H // dve_split
                    dve_slices = [slice(d * h_c, (d + 1) * h_c)
                                  for d in range(dve_split)]
                for d, hs in enumerate(dve_slices):
                    if inplace:
                        dst = ov[:, hs]
                    else:
                        junk = jpool.tile([NCELL, hs.stop - hs.start], f32,
                                          tag="junk")
                        dst = junk[:]
                    nc.vector.scalar_tensor_tensor(
                        dst, ov[:, hs], 1.0,
                        at[:, img * H + hs.start:img * H + hs.stop],
                        mybir.AluOpType.mult, mybir.AluOpType.mult,
                        accum_out=macc[:, 2 * img + d:2 * img + d + 1])

        nc.sync.dma_start(out_d[:], macc[:])

    nc.compile()
    _compiled[key] = nc
    return nc


def _cells_and_weights(boxes_img):
    """Per-image elementary x-cells + {1,13} weight matrix.
    boxes_img: [32, 4] f32. Returns C [W, NCELL] f32 0/1, A [NCELL, H] f32."""
    u1 = np.floor(boxes_img[:, 0]).astype(np.int64)
    v1 = np.floor(boxes_img[:, 1]).astype(np.int64)
    u2 = np.ceil(boxes_img[:, 2]).astype(np.int64)
    v2 = np.ceil(boxes_img[:, 3]).astype(np.int64)
    bps = np.unique(np.clip(np.concatenate([[0, W], u1, u2]), 0, W))
    ca, cb = bps[:-1], bps[1:]           # cells [ca, cb), m <= 65
    m = len(ca)
    C = np.zeros((W, NCELL), np.float32)
    w_idx = np.arange(W)[:, None]
    C[:, :m] = ((w_idx >= ca[None, :]) & (w_idx < cb[None, :]))
    hh = np.arange(H)[None, :]
    row_in = (hh >= v1[:, None]) & (hh < v2[:, None])          # [32, H]
    cell_in = (ca[:, None] >= u1[None, :]) & (cb[:, None] <= u2[None, :])
    fg = (cell_in[:, :, None] & row_in[None, :, :]).any(1)     # [m, H]
    A = np.zeros((NCELL, H), np.float32)
    A[:m] = 1.0 + 12.0 * fg
    return C, A


def _make_in_maps(loss, gt_boxes2d, layout="std", dma_split=2):
    loss = np.asarray(loss, dtype=np.float32)
    boxes = np.asarray(gt_boxes2d, dtype=np.float32).reshape(B, N_PER_IMG, 4)
    lq = loss.astype(np_fp8)
    maps = []
    for c in range(N_CORES):
        # [i, h, w] -> [p, i, t, h] w-tile-major transposed layout
        sh = lq[c * IMGS:(c + 1) * IMGS]                 # [2, H, W]
        sh = sh.transpose(0, 2, 1).reshape(IMGS, WT, P, H)
        if layout == "cb":
            # [(i,t-chunk), p, t-in-chunk, h] chunk-contiguous blocks
            nch = IMGS * dma_split
            wt_c = WT // dma_split
            sh = sh.reshape(nch, wt_c, P, H).transpose(0, 2, 1, 3)
            sh = np.ascontiguousarray(sh).reshape(nch, P, wt_c * H)
        else:
            sh = np.ascontiguousarray(sh.transpose(2, 0, 1, 3)).reshape(P, -1)
        cms, ams = [], []
        for i in range(IMGS):
            C, A = _cells_and_weights(boxes[c * IMGS + i])
            Cp = np.zeros((W, NCP), np.float32)
            Cp[:, :NCELL] = C
            cms.append(Cp.reshape(WT, P, NCP))           # [t, p, j]
            ams.append(A)
        cm = np.stack(cms, 0).transpose(2, 0, 1, 3)      # [p, i, t, j]
        cm = np.ascontiguousarray(cm).reshape(P, -1).astype(np_fp8)
        am = np.ascontiguousarray(np.concatenate(ams, axis=1))  # [65, 2*H]
        maps.append({"loss": sh, "cmat": cm, "amat": am})
    return maps


def kernel(loss, gt_boxes2d, num_gt_per_img=N_PER_IMG):
    nc = _build()
    in_maps = _make_in_maps(loss, gt_boxes2d)
    r = run_bass_kernel_spmd(nc, in_maps, list(range(N_CORES)))
    s = 0.0
    for c in range(N_CORES):
        s += float(np.sum(r.results[c]["out"], dtype=np.float64))
    return np.float32(s / float(B * H * W))
